# revision 1
# baseline (speedup 1.0000x reference)
"""KWinners2d top-k masking kernel for Trainium2 (8 NeuronCores, batch-parallel).

Algorithm (per sample, n = 256*32*32 = 262144, k = 26214):
  boosted y = x * boost[c];  T = k-th largest of y;  out = x * (y >= T).

Exact k-th largest selection on device, per sample:
  1. y = boost_c * x          (ACT, per-partition scale, exact f32 mult)
  2. c0 ~= #{y >= u0}         (ACT Sign + accumulator; +-1 error harmless)
     u0 = build-time quantile of the boosted mixture at tail prob k/n.
  3. u1 = u0 + (c0-(k-300))/(n*pdf)  so that c(u1) ~= k-300 (sub-sample-exact
     Newton step using the true mixture density).  u2 = u1 - 700/(n*pdf).
  4. exact c1 = #{y >= u1} and band count B = #{u2 <= y < u1}  (fused DVE
     tensor_scalar / scalar_tensor_tensor passes with accumulators)
  5. zz = y where in band else -1e30, plus P = 16*(k-c1) - 31 - B pad slots
     of -1e28 (valid, below band).  GPSIMD kth_largest with quantile 15/16
     then computes k_adj = (B+P-1)//16 = r-2 and returns desc[r-1] = exact
     global k-th largest T (r = k - c1 = rank of T within the band).
  6. out = (y >= T) * x       (fused DVE pass)

The pipeline is exact: every count uses exact f32 compares, the band is
guaranteed (prob < 1e-6 otherwise, checked host-side via the stats output
with a numpy fallback per offending sample) to contain rank k with
r in [2,508] so the GPSIMD heap (cap 510) suffices.
"""

import math
from contextlib import ExitStack

import numpy as np

B_FULL = 128
N_CORES = 8
BS = B_FULL // N_CORES          # samples per core
C = 256
HW = 1024                       # 32*32
N = C * HW                      # per-sample elements
K = int(round(N * 0.1))         # 26214
NPAD = 64                       # pad columns in zz
NPL = 2 * HW + NPAD             # kth_largest n_per_lane = 2112
TARGET_GAP = 300.0              # c(u1) target = K - TARGET_GAP
BAND_RANKS = 700.0              # target band width in ranks
VALID_PAD = -1.0e28             # > -1e29  -> counted valid by kth_largest
INVALID = -1.0e30               # < -1e29  -> ignored by kth_largest

_CACHE: dict[bytes, tuple] = {}
TRACE = False          # set True to capture an NTFF profile in LAST_RESULTS
LAST_RESULTS = None


def _mixture_consts(boost: np.ndarray):
    """u0 with P(|mixture| tail >= u0) = K/N, and pdf at u0, for the
    boosted mixture  y ~ (1/C) sum_c N(0, boost_c^2)."""
    b = boost.astype(np.float64)
    target = K / N

    def tail(u):  # P(Y >= u)
        return float(np.mean(0.5 * np.vectorize(math.erfc)(u / (b * math.sqrt(2.0)))))

    lo, hi = 0.0, 20.0
    for _ in range(80):
        mid = 0.5 * (lo + hi)
        if tail(mid) > target:
            lo = mid
        else:
            hi = mid
    u0 = 0.5 * (lo + hi)
    pdf = float(
        np.mean(np.exp(-0.5 * (u0 / b) ** 2) / (b * math.sqrt(2.0 * math.pi)))
    )
    return u0, pdf


def _build(boost: np.ndarray):
    import concourse.bass as bass
    import concourse.mybir as mybir
    from concourse.tile import TileContext

    fp = mybir.dt.float32
    Alu = mybir.AluOpType
    Act = mybir.ActivationFunctionType

    u0, pdf = _mixture_consts(boost)
    inv = 1.0 / (N * pdf)               # value-units per rank
    slope = inv / 2.0
    icept = u0 + (N / 2.0 - K + TARGET_GAP) * inv
    c2 = BAND_RANKS * inv               # u2 = u1 - c2

    import concourse.bacc as bacc
    nc = bacc.Bacc("TRN2", target_bir_lowering=False, debug=False,
                   num_devices=N_CORES)

    x_d = nc.dram_tensor("x", [BS, C, HW], fp, kind="ExternalInput").ap()
    boost_d = nc.dram_tensor("boost", [C, 1], fp, kind="ExternalInput").ap()
    iota_d = nc.dram_tensor("iota", [128, NPAD], fp, kind="ExternalInput").ap()
    out_d = nc.dram_tensor("out", [BS, C, HW], fp, kind="ExternalOutput").ap()
    st_d = nc.dram_tensor("stats", [BS, 8], fp, kind="ExternalOutput").ap()

    from concourse import library_config

    es = ExitStack()
    with TileContext(nc) as tc, es:
        nc.gpsimd.load_library(library_config.attn)
        cpool = es.enter_context(tc.tile_pool(name="const", bufs=1))
        xpool = es.enter_context(tc.tile_pool(name="x", bufs=2))
        ypool = es.enter_context(tc.tile_pool(name="y", bufs=2))
        tpool = es.enter_context(tc.tile_pool(name="t", bufs=2))
        opool = es.enter_context(tc.tile_pool(name="o", bufs=2))
        zpool = es.enter_context(tc.tile_pool(name="z", bufs=2))
        spool = es.enter_context(tc.tile_pool(name="s", bufs=3))
        ppool = es.enter_context(tc.tile_pool(name="ps", bufs=1, space="PSUM"))

        boost_t = cpool.tile([128, 2], fp, tag="boost")
        nc.sync.dma_start(boost_t[:, 0:1], boost_d[0:128, :])
        nc.sync.dma_start(boost_t[:, 1:2], boost_d[128:256, :])
        iota_t = cpool.tile([128, NPAD], fp, tag="iota")
        nc.sync.dma_start(iota_t, iota_d)
        padval = cpool.tile([128, NPAD], fp, tag="padval")
        nc.vector.memset(padval, VALID_PAD)
        onesT = cpool.tile([128, 1], fp, tag="onesT")   # lhsT for col sums
        nc.vector.memset(onesT, 1.0)
        ones1 = cpool.tile([1, 128], fp, tag="ones1")   # lhsT for broadcast
        nc.vector.memset(ones1, 1.0)
        scr = cpool.tile([128, HW], fp, tag="scr")      # sign-output scratch
        negu0 = cpool.tile([128, 1], fp, tag="negu0")
        nc.vector.memset(negu0, -u0)

        for s in range(BS):
            xa = xpool.tile([128, HW], fp, tag="xa")
            xb = xpool.tile([128, HW], fp, tag="xb")
            nc.sync.dma_start(xa, x_d[s, 0:128, :])
            nc.sync.dma_start(xb, x_d[s, 128:256, :])

            ya = ypool.tile([128, HW], fp, tag="ya")
            yb = ypool.tile([128, HW], fp, tag="yb")
            nc.scalar.mul(ya, xa, boost_t[:, 0:1])
            nc.scalar.mul(yb, xb, boost_t[:, 1:2])

            # --- coarse count via sign-sum at u0 ---------------------------
            sgn = spool.tile([128, 2], fp, tag="sgn")
            nc.scalar.activation(scr, ya, Act.Sign, bias=negu0[:, 0:1],
                                 accum_out=sgn[:, 0:1])
            nc.scalar.activation(scr, yb, Act.Sign, bias=negu0[:, 0:1],
                                 accum_out=sgn[:, 1:2])
            psS = ppool.tile([1, 1], fp, tag="psS")
            nc.tensor.matmul(psS, onesT, sgn[:, 0:1], start=True, stop=False)
            nc.tensor.matmul(psS, onesT, sgn[:, 1:2], start=False, stop=True)

            # u1 = slope*S + icept ; u2 = u1 - c2   (packed [1,2])
            u12s = spool.tile([1, 2], fp, tag="u12s")
            nc.vector.tensor_scalar(u12s[0:1, 0:1], psS, slope, icept,
                                    op0=Alu.mult, op1=Alu.add)
            nc.vector.tensor_scalar(u12s[0:1, 1:2], u12s[0:1, 0:1], -c2, None,
                                    op0=Alu.add)
            psU = ppool.tile([128, 2], fp, tag="psU")
            nc.tensor.matmul(psU, ones1, u12s, start=True, stop=True)
            u12 = spool.tile([128, 2], fp, tag="u12")
            nc.vector.tensor_copy(u12, psU)

            # --- exact c1 and band count B ---------------------------------
            ta = tpool.tile([128, HW], fp, tag="ta")
            tb = tpool.tile([128, HW], fp, tag="tb")
            fa = tpool.tile([128, HW], mybir.dt.uint8, tag="fa")
            fb = tpool.tile([128, HW], mybir.dt.uint8, tag="fb")
            acc = spool.tile([128, 4], fp, tag="acc")
            nc.vector.tensor_scalar(ta, ya, u12[:, 0:1], None, op0=Alu.is_ge,
                                    op1=Alu.add, accum_out=acc[:, 0:1])
            nc.vector.tensor_scalar(tb, yb, u12[:, 0:1], None, op0=Alu.is_ge,
                                    op1=Alu.add, accum_out=acc[:, 1:2])
            nc.vector.scalar_tensor_tensor(fa, ya, u12[:, 1:2], ta,
                                           op0=Alu.is_ge, op1=Alu.subtract,
                                           accum_out=acc[:, 2:3])
            nc.vector.scalar_tensor_tensor(fb, yb, u12[:, 1:2], tb,
                                           op0=Alu.is_ge, op1=Alu.subtract,
                                           accum_out=acc[:, 3:4])
            psA = ppool.tile([1, 2], fp, tag="psA")     # [c1, B]
            nc.tensor.matmul(psA, onesT, acc[:, 0:4:2], start=True, stop=False)
            nc.tensor.matmul(psA, onesT, acc[:, 1:4:2], start=False, stop=True)

            # r = clamp(K - c1, 2, 508) ; P = 16r - B - 31 (>= 0)
            rP = spool.tile([1, 2], fp, tag="rP")
            nc.vector.tensor_scalar(rP[0:1, 0:1], psA[0:1, 0:1], -1.0, float(K),
                                    op0=Alu.mult, op1=Alu.add)
            nc.vector.tensor_scalar(rP[0:1, 0:1], rP[0:1, 0:1], 2.0, 508.0,
                                    op0=Alu.max, op1=Alu.min)
            nc.vector.scalar_tensor_tensor(rP[0:1, 1:2], rP[0:1, 0:1], 16.0,
                                           psA[0:1, 1:2],
                                           op0=Alu.mult, op1=Alu.subtract)
            nc.vector.tensor_scalar(rP[0:1, 1:2], rP[0:1, 1:2], -31.0, 0.0,
                                    op0=Alu.add, op1=Alu.max)
            psP = ppool.tile([128, 1], fp, tag="psP")
            nc.tensor.matmul(psP, ones1, rP[0:1, 1:2], start=True, stop=True)

            # --- zz: band values + P valid pads ---------------------------
            zz = zpool.tile([128, NPL], fp, tag="zz")
            nc.gpsimd.memset(zz, INVALID)
            nc.vector.copy_predicated(zz[:, 0:HW], fa, ya)
            nc.vector.copy_predicated(zz[:, HW:2 * HW], fb, yb)
            pm = spool.tile([128, NPAD], mybir.dt.uint8, tag="pm")
            nc.vector.tensor_scalar(pm, iota_t, psP, None, op0=Alu.is_lt)
            nc.vector.copy_predicated(zz[:, 2 * HW:], pm, padval)

            kout = spool.tile([1, 2], fp, tag="kout")
            nc.gpsimd.kth_largest(kout, zz, n_per_lane=NPL, k=510,
                                  quantile=1.0 - 1.0 / 16.0)

            psT = ppool.tile([128, 1], fp, tag="psT")
            nc.tensor.matmul(psT, ones1, kout[0:1, 1:2], start=True, stop=True)
            Tb = spool.tile([128, 1], fp, tag="Tb")
            nc.vector.tensor_copy(Tb, psT)

            # --- final mask ------------------------------------------------
            oa = opool.tile([128, HW], fp, tag="oa")
            ob = opool.tile([128, HW], fp, tag="ob")
            nc.vector.scalar_tensor_tensor(oa, ya, Tb, xa,
                                           op0=Alu.is_ge, op1=Alu.mult)
            nc.vector.scalar_tensor_tensor(ob, yb, Tb, xb,
                                           op0=Alu.is_ge, op1=Alu.mult)
            nc.sync.dma_start(out_d[s, 0:128, :], oa)
            nc.sync.dma_start(out_d[s, 128:256, :], ob)

            nc.sync.dma_start(st_d[s:s + 1, 2:4], rP)        # r, P
            nc.sync.dma_start(st_d[s:s + 1, 4:6], kout)      # lerp, T

    nc.compile()
    return nc


def _get_program(boost: np.ndarray):
    key = boost.tobytes()
    if key not in _CACHE:
        _CACHE[key] = _build(boost)
    return _CACHE[key]


def _boost_from_duty(dutyCycle: np.ndarray) -> np.ndarray:
    # computed with jax-on-CPU to bit-match the reference's jnp.exp
    import jax
    import jax.numpy as jnp
    target_density = float(K) / float(N)
    cpu = jax.devices("cpu")[0]
    with jax.default_device(cpu):
        d = jax.device_put(np.asarray(dutyCycle), cpu)
        boost = jnp.exp((target_density - d) * 1.0)
    return np.asarray(boost, dtype=np.float32).reshape(C)


def kernel(x: np.ndarray, dutyCycle: np.ndarray) -> np.ndarray:
    from concourse import bass_utils

    x = np.ascontiguousarray(x, dtype=np.float32)
    boost = _boost_from_duty(dutyCycle)
    nc = _get_program(boost)

    xr = x.reshape(N_CORES, BS, C, HW)
    boost_in = boost.reshape(C, 1)
    iota_in = (np.arange(128 * NPAD, dtype=np.float32)
               .reshape(128, NPAD))
    in_maps = [{"x": xr[c], "boost": boost_in, "iota": iota_in}
               for c in range(N_CORES)]
    try:
        res = bass_utils.run_bass_kernel_spmd(nc, in_maps,
                                              core_ids=list(range(N_CORES)),
                                              trace=TRACE)
    except ModuleNotFoundError:
        # no NTFF profiling hook in this container — run untraced
        res = bass_utils.run_bass_kernel_spmd(nc, in_maps,
                                              core_ids=list(range(N_CORES)))
    global LAST_RESULTS
    LAST_RESULTS = res
    out = np.concatenate([res.results[c]["out"][None] for c in range(N_CORES)])
    out = out.reshape(B_FULL, C, 32, 32)
    stats = np.concatenate([res.results[c]["stats"][None]
                            for c in range(N_CORES)]).reshape(B_FULL, 8)

    # host-side validity guard (prob ~1e-6); numpy fallback per bad sample.
    # r,P were clamped on device; clamp-bound values mark invalid samples.
    r, P = stats[:, 2], stats[:, 3]
    B = 16.0 * r - 31.0 - P
    bad = (r <= 2) | (r >= 508) | (P <= 0) | (P > 8191) | (r > B)
    if bad.any():
        for s in np.nonzero(bad)[0]:
            boosted = (x[s].reshape(C, HW) * boost[:, None]).ravel()
            thr = np.partition(boosted, N - K)[N - K]
            out[s] = (x[s].reshape(C, HW)
                      * (boosted.reshape(C, HW) >= thr)).reshape(C, 32, 32)
    return out



# revision 9
# speedup vs baseline: 1.5014x; 1.5014x over previous
"""KWinners2d top-k masking kernel for Trainium2 (8 NeuronCores, batch-parallel).

Device algorithm (per sample, n = 256*32*32 = 262144, k = 26214):
  boosted y = x * boost[c];  T = k-th largest of y;  mask = (y >= T).

Exact k-th largest selection on device, per sample:
  1. y = boost_c * x          (ACT, per-partition scale, exact f32 mult)
  2. c0 ~= #{y >= u0}         (ACT Sign + accumulator; +-1 error harmless)
     u0 = build-time quantile of the boosted mixture at tail prob k/n.
  3. u1 = u0 + (c0-(k-300))/(n*pdf)  so that c(u1) ~= k-300 (sub-sample-exact
     Newton step using the true mixture density).  u2 = u1 - 700/(n*pdf).
  4. exact c1 = #{y >= u1} and band count B = #{u2 <= y < u1}  (fused DVE
     tensor_scalar / scalar_tensor_tensor passes with accumulators)
  5. zz = y where in band else -1e30, plus P = 16*(k-c1) - 31 - B pad slots
     of -1e28 (valid, below band).  GPSIMD kth_largest with quantile 15/16
     then computes k_adj = (B+P-1)//16 = r-2 and returns desc[r-1] = exact
     global k-th largest T (r = k - c1 = rank of T within the band).
  6. mask bits = (y >= T), packed 8-per-byte on device (DVE strided
     scale-adds), downloaded as uint8 [BS, C, HW/8].

The pipeline is exact: every count uses exact f32 compares, the band is
guaranteed (prob < 1e-6 otherwise, checked host-side via the stats output
with a numpy fallback per offending sample) to contain rank k with
r in [2,508] so the GPSIMD heap (cap 510) suffices.

Host side applies out = x * mask (a fused multithreaded XLA-CPU pass).
Only x (128 MB) is uploaded and the 4 MB packed mask downloaded; the
jitted PJRT executable is built once and cached, so steady-state wall
time is dominated by the unavoidable x upload over the axon tunnel.
"""

import math
from contextlib import ExitStack

import numpy as np

B_FULL = 128
N_CORES = 8
BS = B_FULL // N_CORES          # samples per core
C = 256
HW = 1024                       # 32*32
N = C * HW                      # per-sample elements
K = int(round(N * 0.1))         # 26214
NPAD = 64                       # pad columns in zz
NPL = 2 * HW + NPAD             # kth_largest n_per_lane = 2112
TARGET_GAP = 300.0              # c(u1) target = K - TARGET_GAP
BAND_RANKS = 700.0              # target band width in ranks
VALID_PAD = -1.0e28             # > -1e29  -> counted valid by kth_largest
INVALID = -1.0e30               # < -1e29  -> ignored by kth_largest

_CACHE: dict[bytes, tuple] = {}   # boost bytes -> (nc, exec handle)
_BOOST_CACHE: dict[bytes, np.ndarray] = {}
_APPLY_JIT = None
TRACE = False
LAST_RESULTS = None


def _mixture_consts(boost: np.ndarray):
    """u0 with P(|mixture| tail >= u0) = K/N, and pdf at u0, for the
    boosted mixture  y ~ (1/C) sum_c N(0, boost_c^2)."""
    b = boost.astype(np.float64)
    target = K / N

    def tail(u):  # P(Y >= u)
        return float(np.mean(0.5 * np.vectorize(math.erfc)(u / (b * math.sqrt(2.0)))))

    lo, hi = 0.0, 20.0
    for _ in range(80):
        mid = 0.5 * (lo + hi)
        if tail(mid) > target:
            lo = mid
        else:
            hi = mid
    u0 = 0.5 * (lo + hi)
    pdf = float(
        np.mean(np.exp(-0.5 * (u0 / b) ** 2) / (b * math.sqrt(2.0 * math.pi)))
    )
    return u0, pdf


def _build(boost: np.ndarray):
    import concourse.bass as bass
    import concourse.mybir as mybir
    from concourse.tile import TileContext

    fp = mybir.dt.float32
    Alu = mybir.AluOpType
    Act = mybir.ActivationFunctionType

    u0, pdf = _mixture_consts(boost)
    inv = 1.0 / (N * pdf)               # value-units per rank
    slope = inv / 2.0
    icept = u0 + (N / 2.0 - K + TARGET_GAP) * inv
    c2 = BAND_RANKS * inv               # u2 = u1 - c2

    import concourse.bacc as bacc
    nc = bacc.Bacc("TRN2", target_bir_lowering=False, debug=False,
                   num_devices=N_CORES)

    x_d = nc.dram_tensor("x", [BS, C, HW], fp, kind="ExternalInput").ap()
    boost_d = nc.dram_tensor("boost", [C, 1], fp, kind="ExternalInput").ap()
    iota_d = nc.dram_tensor("iota", [128, NPAD], fp, kind="ExternalInput").ap()
    mask_d = nc.dram_tensor("mask", [BS, C, HW // 8], mybir.dt.uint8,
                            kind="ExternalOutput").ap()
    st_d = nc.dram_tensor("stats", [BS, 8], fp, kind="ExternalOutput").ap()

    from concourse import library_config

    es = ExitStack()
    with TileContext(nc) as tc, es:
        nc.gpsimd.load_library(library_config.attn)
        cpool = es.enter_context(tc.tile_pool(name="const", bufs=1))
        xpool = es.enter_context(tc.tile_pool(name="x", bufs=2))
        ypool = es.enter_context(tc.tile_pool(name="y", bufs=2))
        tpool = es.enter_context(tc.tile_pool(name="t", bufs=2))
        opool = es.enter_context(tc.tile_pool(name="o", bufs=2))
        zpool = es.enter_context(tc.tile_pool(name="z", bufs=2))
        spool = es.enter_context(tc.tile_pool(name="s", bufs=3))
        ppool = es.enter_context(tc.tile_pool(name="ps", bufs=1, space="PSUM"))

        boost_t = cpool.tile([128, 2], fp, tag="boost")
        nc.sync.dma_start(boost_t[:, 0:1], boost_d[0:128, :])
        nc.sync.dma_start(boost_t[:, 1:2], boost_d[128:256, :])
        iota_t = cpool.tile([128, NPAD], fp, tag="iota")
        nc.sync.dma_start(iota_t, iota_d)
        padval = cpool.tile([128, NPAD], fp, tag="padval")
        nc.vector.memset(padval, VALID_PAD)
        onesT = cpool.tile([128, 1], fp, tag="onesT")   # lhsT for col sums
        nc.vector.memset(onesT, 1.0)
        ones1 = cpool.tile([1, 128], fp, tag="ones1")   # lhsT for broadcast
        nc.vector.memset(ones1, 1.0)
        scr = cpool.tile([128, HW], fp, tag="scr")      # sign-output scratch
        negu0 = cpool.tile([128, 1], fp, tag="negu0")
        nc.vector.memset(negu0, -u0)

        for s in range(BS):
            xa = xpool.tile([128, HW], fp, tag="xa")
            xb = xpool.tile([128, HW], fp, tag="xb")
            nc.sync.dma_start(xa, x_d[s, 0:128, :])
            nc.sync.dma_start(xb, x_d[s, 128:256, :])

            ya = ypool.tile([128, HW], fp, tag="ya")
            yb = ypool.tile([128, HW], fp, tag="yb")
            nc.scalar.mul(ya, xa, boost_t[:, 0:1])
            nc.scalar.mul(yb, xb, boost_t[:, 1:2])

            # --- coarse count via sign-sum at u0 ---------------------------
            sgn = spool.tile([128, 2], fp, tag="sgn")
            nc.scalar.activation(scr, ya, Act.Sign, bias=negu0[:, 0:1],
                                 accum_out=sgn[:, 0:1])
            nc.scalar.activation(scr, yb, Act.Sign, bias=negu0[:, 0:1],
                                 accum_out=sgn[:, 1:2])
            psS = ppool.tile([1, 1], fp, tag="psS")
            nc.tensor.matmul(psS, onesT, sgn[:, 0:1], start=True, stop=False)
            nc.tensor.matmul(psS, onesT, sgn[:, 1:2], start=False, stop=True)

            # u1 = slope*S + icept ; u2 = u1 - c2   (packed [1,2])
            u12s = spool.tile([1, 2], fp, tag="u12s")
            nc.vector.tensor_scalar(u12s[0:1, 0:1], psS, slope, icept,
                                    op0=Alu.mult, op1=Alu.add)
            nc.vector.tensor_scalar(u12s[0:1, 1:2], u12s[0:1, 0:1], -c2, None,
                                    op0=Alu.add)
            psU = ppool.tile([128, 2], fp, tag="psU")
            nc.tensor.matmul(psU, ones1, u12s, start=True, stop=True)
            u12 = spool.tile([128, 2], fp, tag="u12")
            nc.vector.tensor_copy(u12, psU)

            # --- exact c1 and band count B ---------------------------------
            ta = tpool.tile([128, HW], fp, tag="ta")
            tb = tpool.tile([128, HW], fp, tag="tb")
            fa = tpool.tile([128, HW], mybir.dt.uint8, tag="fa")
            fb = tpool.tile([128, HW], mybir.dt.uint8, tag="fb")
            acc = spool.tile([128, 4], fp, tag="acc")
            nc.vector.tensor_scalar(ta, ya, u12[:, 0:1], None, op0=Alu.is_ge,
                                    op1=Alu.add, accum_out=acc[:, 0:1])
            nc.vector.tensor_scalar(tb, yb, u12[:, 0:1], None, op0=Alu.is_ge,
                                    op1=Alu.add, accum_out=acc[:, 1:2])
            nc.vector.scalar_tensor_tensor(fa, ya, u12[:, 1:2], ta,
                                           op0=Alu.is_ge, op1=Alu.subtract,
                                           accum_out=acc[:, 2:3])
            nc.vector.scalar_tensor_tensor(fb, yb, u12[:, 1:2], tb,
                                           op0=Alu.is_ge, op1=Alu.subtract,
                                           accum_out=acc[:, 3:4])
            psA = ppool.tile([1, 2], fp, tag="psA")     # [c1, B]
            nc.tensor.matmul(psA, onesT, acc[:, 0:4:2], start=True, stop=False)
            nc.tensor.matmul(psA, onesT, acc[:, 1:4:2], start=False, stop=True)

            # r = clamp(K - c1, 2, 508) ; P = 16r - B - 31 (>= 0)
            rP = spool.tile([1, 2], fp, tag="rP")
            nc.vector.tensor_scalar(rP[0:1, 0:1], psA[0:1, 0:1], -1.0, float(K),
                                    op0=Alu.mult, op1=Alu.add)
            nc.vector.tensor_scalar(rP[0:1, 0:1], rP[0:1, 0:1], 2.0, 508.0,
                                    op0=Alu.max, op1=Alu.min)
            nc.vector.scalar_tensor_tensor(rP[0:1, 1:2], rP[0:1, 0:1], 16.0,
                                           psA[0:1, 1:2],
                                           op0=Alu.mult, op1=Alu.subtract)
            nc.vector.tensor_scalar(rP[0:1, 1:2], rP[0:1, 1:2], -31.0, 0.0,
                                    op0=Alu.add, op1=Alu.max)
            psP = ppool.tile([128, 1], fp, tag="psP")
            nc.tensor.matmul(psP, ones1, rP[0:1, 1:2], start=True, stop=True)

            # --- zz: band values + P valid pads ---------------------------
            zz = zpool.tile([128, NPL], fp, tag="zz")
            nc.gpsimd.memset(zz, INVALID)
            nc.vector.copy_predicated(zz[:, 0:HW], fa, ya)
            nc.vector.copy_predicated(zz[:, HW:2 * HW], fb, yb)
            pm = spool.tile([128, NPAD], mybir.dt.uint8, tag="pm")
            nc.vector.tensor_scalar(pm, iota_t, psP, None, op0=Alu.is_lt)
            nc.vector.copy_predicated(zz[:, 2 * HW:], pm, padval)

            kout = spool.tile([1, 2], fp, tag="kout")
            nc.gpsimd.kth_largest(kout, zz, n_per_lane=NPL, k=510,
                                  quantile=1.0 - 1.0 / 16.0)

            psT = ppool.tile([128, 1], fp, tag="psT")
            nc.tensor.matmul(psT, ones1, kout[0:1, 1:2], start=True, stop=True)
            Tb = spool.tile([128, 1], fp, tag="Tb")
            nc.vector.tensor_copy(Tb, psT)

            # --- final mask, packed 8 bits/byte ---------------------------
            ma = opool.tile([128, HW], fp, tag="ma")
            mb = opool.tile([128, HW], fp, tag="mb")
            nc.vector.tensor_scalar(ma, ya, Tb, None, op0=Alu.is_ge)
            nc.vector.tensor_scalar(mb, yb, Tb, None, op0=Alu.is_ge)
            # pack: byte m = sum_i mask[8m+i] * 2^i  (little-endian bits)
            pk0 = opool.tile([128, 256], fp, tag="pk0")
            pk1 = opool.tile([128, 256], fp, tag="pk1")
            for h, m in ((0, ma), (1, mb)):
                lo = 128 * h
                cur, alt = pk0[:, lo:lo + 128], pk1[:, lo:lo + 128]
                nc.vector.tensor_scalar(cur, m[:, 0:HW:8], 1.0, None,
                                        op0=Alu.mult)
                for i in range(1, 8):
                    nc.vector.scalar_tensor_tensor(alt, m[:, i:HW:8],
                                                   float(1 << i), cur,
                                                   op0=Alu.mult, op1=Alu.add)
                    cur, alt = alt, cur
            # 7 swaps -> final accumulation lives in pk1
            u8 = opool.tile([128, 256], mybir.dt.uint8, tag="u8")
            nc.vector.tensor_copy(u8, pk1)
            nc.sync.dma_start(mask_d[s, 0:128, :], u8[:, 0:128])
            nc.sync.dma_start(mask_d[s, 128:256, :], u8[:, 128:256])

            nc.sync.dma_start(st_d[s:s + 1, 2:4], rP)        # r, P
            nc.sync.dma_start(st_d[s:s + 1, 4:6], kout)      # lerp, T

    nc.compile()
    return nc


class _Exec:
    """Cached PJRT executable for the SPMD bass program.

    Replicates concourse.bass2jax.run_bass_via_pjrt's multi-core path but
    builds the jitted shard_map once and reuses it, and assembles global
    inputs without host-side concatenation copies.
    """

    def __init__(self, nc, boost: np.ndarray):
        import jax
        import jax.numpy as jnp
        from jax.experimental.shard_map import shard_map
        from jax.sharding import Mesh, NamedSharding, PartitionSpec

        import concourse.mybir as mybir
        from concourse import bass2jax

        bass2jax.install_neuronx_cc_hook()
        self.nc = nc
        if nc.dbg_callbacks:
            raise RuntimeError("dbg_callbacks unsupported on the axon client")

        partition_name = (nc.partition_id_tensor.name
                          if nc.partition_id_tensor else None)
        in_names: list[str] = []
        out_names: list[str] = []
        out_avals = []
        zero_shapes: list[tuple[tuple, np.dtype]] = []
        for alloc in nc.m.functions[0].allocations:
            if not isinstance(alloc, mybir.MemoryLocationSet):
                continue
            name = alloc.memorylocations[0].name
            if alloc.kind == "ExternalInput":
                if name != partition_name:
                    in_names.append(name)
            elif alloc.kind == "ExternalOutput":
                shape = tuple(alloc.tensor_shape)
                dtype = mybir.dt.np(alloc.dtype)
                out_names.append(name)
                out_avals.append(jax.core.ShapedArray(shape, dtype))
                zero_shapes.append(
                    ((N_CORES * shape[0], *shape[1:]), np.dtype(dtype)))
        n_params = len(in_names)
        all_names = list(in_names) + out_names
        if partition_name is not None:
            all_names.append(partition_name)

        def _body(*args):
            operands = list(args)
            if partition_name is not None:
                operands.append(bass2jax.partition_id_tensor())
            outs = bass2jax._bass_exec_p.bind(
                *operands,
                out_avals=tuple(out_avals),
                in_names=tuple(all_names),
                out_names=tuple(out_names),
                lowering_input_output_aliases=(),
                sim_require_finite=True,
                sim_require_nnan=True,
                nc=nc,
            )
            return tuple(outs)

        devices = jax.devices()[:N_CORES]
        assert len(devices) == N_CORES
        mesh = Mesh(np.asarray(devices), ("core",))
        n_outs = len(out_names)
        donate = tuple(range(n_params, n_params + n_outs))
        self.fn = jax.jit(
            shard_map(_body, mesh=mesh,
                      in_specs=(PartitionSpec("core"),) * (n_params + n_outs),
                      out_specs=(PartitionSpec("core"),) * n_outs,
                      check_rep=False),
            donate_argnums=donate,
            keep_unused=True,
        )
        self.in_names = in_names
        self.out_names = out_names
        self.sharding = NamedSharding(mesh, PartitionSpec("core"))
        # constant inputs committed to device once; reused every call
        dbg_name = nc.dbg_addr.name if nc.dbg_addr is not None else None
        consts = {
            "boost": np.tile(boost.reshape(1, C, 1),
                             (N_CORES, 1, 1)).reshape(N_CORES * C, 1),
            "iota": np.tile(
                np.arange(128 * NPAD, dtype=np.float32).reshape(1, 128, NPAD),
                (N_CORES, 1, 1)).reshape(N_CORES * 128, NPAD),
        }
        if dbg_name is not None:
            consts[dbg_name] = np.zeros((N_CORES, 2), np.uint32)
        self.d_consts = {n: jax.device_put(v, self.sharding)
                         for n, v in consts.items()}
        # donated output buffers created on device (no host upload)
        mk = [(shp, jnp.dtype(dt)) for shp, dt in zero_shapes]
        self.mkz = jax.jit(
            lambda: tuple(jnp.zeros(shp, dt) for shp, dt in mk),
            out_shardings=(self.sharding,) * n_outs,
        )

    def run(self, x_global: np.ndarray):
        """Dispatch the NEFF; returns the raw (async) jax output arrays."""
        import jax
        d_x = jax.device_put(x_global, self.sharding)   # async upload starts
        zeros = self.mkz()                              # created on device
        args = [d_x if n == "x" else self.d_consts[n] for n in self.in_names]
        args += list(zeros)
        outs = self.fn(*args)
        return {n: outs[i] for i, n in enumerate(self.out_names)}


def _get_program(boost: np.ndarray):
    key = boost.tobytes()
    if key not in _CACHE:
        nc = _build(boost)
        _CACHE[key] = (nc, _Exec(nc, boost))
    return _CACHE[key]


def _boost_from_duty(dutyCycle: np.ndarray) -> np.ndarray:
    # computed with jax-on-CPU to bit-match the reference's jnp.exp
    duty = np.ascontiguousarray(np.asarray(dutyCycle, dtype=np.float32))
    key = duty.tobytes()
    if key not in _BOOST_CACHE:
        import jax
        import jax.numpy as jnp
        target_density = float(K) / float(N)
        cpu = jax.devices("cpu")[0]
        with jax.default_device(cpu):
            d = jax.device_put(duty, cpu)
            boost = jnp.exp((target_density - d) * 1.0)
        _BOOST_CACHE[key] = np.asarray(boost, dtype=np.float32).reshape(C)
    return _BOOST_CACHE[key]


def _apply_mask(x4: np.ndarray, mask_bytes: np.ndarray) -> np.ndarray:
    """out = x * unpacked(mask) — fused multithreaded pass on XLA CPU."""
    global _APPLY_JIT
    import jax
    import jax.numpy as jnp
    cpu = jax.devices("cpu")[0]
    if _APPLY_JIT is None:
        def _f(x, mb):
            bits = (mb[..., None] >> jnp.arange(8, dtype=jnp.uint8)) \
                & jnp.uint8(1)
            m = bits.reshape(B_FULL, C, 32, 32)
            return x * m.astype(jnp.float32)
        _APPLY_JIT = jax.jit(_f)
    with jax.default_device(cpu):
        out = _APPLY_JIT(jax.device_put(x4, cpu), jax.device_put(mask_bytes, cpu))
        return np.asarray(out)


def _numpy_reference(x: np.ndarray, boost: np.ndarray) -> np.ndarray:
    out = np.empty_like(x)
    xf = x.reshape(B_FULL, C, HW)
    for s in range(B_FULL):
        boosted = (xf[s] * boost[:, None]).ravel()
        thr = np.partition(boosted, N - K)[N - K]
        out[s] = (xf[s] * (boosted.reshape(C, HW) >= thr)).reshape(C, 32, 32)
    return out


def kernel(x: np.ndarray, dutyCycle: np.ndarray) -> np.ndarray:
    x = np.ascontiguousarray(np.asarray(x), dtype=np.float32)
    boost = _boost_from_duty(dutyCycle)
    try:
        nc, ex = _get_program(boost)
        res = ex.run(x.reshape(B_FULL, C, HW))       # reshape is a view
        mask_bytes = np.asarray(res["mask"]).reshape(B_FULL, C, HW // 8)
        try:  # prefetch stats d2h while the host applies the mask
            res["stats"].copy_to_host_async()
        except Exception:
            pass
        out = _apply_mask(x, mask_bytes)
        stats = np.asarray(res["stats"]).reshape(B_FULL, 8)
    except Exception as e:  # safety net: keep correctness if device path dies
        import traceback
        traceback.print_exc()
        print(f"kernel: device path failed ({e!r}); numpy fallback")
        return _numpy_reference(x, boost)
    global LAST_RESULTS
    LAST_RESULTS = type("R", (), {"exec_time_ns": None, "results": res})()

    # host-side validity guard (prob ~1e-6); numpy fallback per bad sample.
    # r,P were clamped on device; clamp-bound values mark invalid samples.
    r, P = stats[:, 2], stats[:, 3]
    B = 16.0 * r - 31.0 - P
    bad = (r <= 2) | (r >= 508) | (P <= 0) | (P > 8191) | (r > B)
    if bad.any():
        if not out.flags.writeable:
            out = out.copy()
        for s in np.nonzero(bad)[0]:
            boosted = (x[s].reshape(C, HW) * boost[:, None]).ravel()
            thr = np.partition(boosted, N - K)[N - K]
            out[s] = (x[s].reshape(C, HW)
                      * (boosted.reshape(C, HW) >= thr)).reshape(C, 32, 32)
    return out


# revision 10
# speedup vs baseline: 2.8966x; 1.9293x over previous
"""KWinners2d top-k masking kernel for Trainium2 (8 NeuronCores, batch-parallel).

Device algorithm (per sample, n = 256*32*32 = 262144, k = 26214):
  boosted y = x * boost[c];  T = k-th largest of y;  mask = (y >= T).

Exact k-th largest selection on device, per sample:
  1. y = boost_c * x          (ACT, per-partition scale, exact f32 mult)
  2. c0 ~= #{y >= u0}         (ACT Sign + accumulator; +-1 error harmless)
     u0 = build-time quantile of the boosted mixture at tail prob k/n.
  3. u1 = u0 + (c0-(k-300))/(n*pdf)  so that c(u1) ~= k-300 (sub-sample-exact
     Newton step using the true mixture density).  u2 = u1 - 700/(n*pdf).
  4. exact c1 = #{y >= u1} and band count B = #{u2 <= y < u1}  (fused DVE
     tensor_scalar / scalar_tensor_tensor passes with accumulators)
  5. zz = y where in band else -1e30, plus P = 16*(k-c1) - 31 - B pad slots
     of -1e28 (valid, below band).  GPSIMD kth_largest with quantile 15/16
     then computes k_adj = (B+P-1)//16 = r-2 and returns desc[r-1] = exact
     global k-th largest T (r = k - c1 = rank of T within the band).
  6. mask bits = (y >= T), packed 8-per-byte on device (DVE strided
     scale-adds), downloaded as uint8 [BS, C, HW/8].

The pipeline is exact: every count uses exact f32 compares, the band is
guaranteed (prob < 1e-6 otherwise, checked host-side via the stats output
with a numpy fallback per offending sample) to contain rank k with
r in [2,508] so the GPSIMD heap (cap 510) suffices.

Host side applies out = x * mask (a fused multithreaded XLA-CPU pass).
Only x (128 MB) is uploaded and the 4 MB packed mask downloaded; the
jitted PJRT executable is built once and cached, so steady-state wall
time is dominated by the unavoidable x upload over the axon tunnel.
"""

import math
from contextlib import ExitStack

import numpy as np

B_FULL = 128
N_CORES = 8
BS = B_FULL // N_CORES          # samples per core
C = 256
HW = 1024                       # 32*32
N = C * HW                      # per-sample elements
K = int(round(N * 0.1))         # 26214
NPAD = 64                       # pad columns in zz
NPL = 2 * HW + NPAD             # kth_largest n_per_lane = 2112
TARGET_GAP = 300.0              # c(u1) target = K - TARGET_GAP
BAND_RANKS = 700.0              # target band width in ranks
VALID_PAD = -1.0e28             # > -1e29  -> counted valid by kth_largest
INVALID = -1.0e30               # < -1e29  -> ignored by kth_largest

_CACHE: dict[bytes, tuple] = {}   # boost bytes -> (nc, exec handle)
_BOOST_CACHE: dict[bytes, np.ndarray] = {}
_APPLY_JIT = None
TRACE = False
LAST_RESULTS = None


def _mixture_consts(boost: np.ndarray):
    """u0 with P(|mixture| tail >= u0) = K/N, and pdf at u0, for the
    boosted mixture  y ~ (1/C) sum_c N(0, boost_c^2)."""
    b = boost.astype(np.float64)
    target = K / N

    def tail(u):  # P(Y >= u)
        return float(np.mean(0.5 * np.vectorize(math.erfc)(u / (b * math.sqrt(2.0)))))

    lo, hi = 0.0, 20.0
    for _ in range(80):
        mid = 0.5 * (lo + hi)
        if tail(mid) > target:
            lo = mid
        else:
            hi = mid
    u0 = 0.5 * (lo + hi)
    pdf = float(
        np.mean(np.exp(-0.5 * (u0 / b) ** 2) / (b * math.sqrt(2.0 * math.pi)))
    )
    return u0, pdf


def _build(boost: np.ndarray):
    import concourse.bass as bass
    import concourse.mybir as mybir
    from concourse.tile import TileContext

    fp = mybir.dt.float32
    Alu = mybir.AluOpType
    Act = mybir.ActivationFunctionType

    u0, pdf = _mixture_consts(boost)
    inv = 1.0 / (N * pdf)               # value-units per rank
    slope = inv / 2.0
    icept = u0 + (N / 2.0 - K + TARGET_GAP) * inv
    c2 = BAND_RANKS * inv               # u2 = u1 - c2

    import concourse.bacc as bacc
    nc = bacc.Bacc("TRN2", target_bir_lowering=False, debug=False,
                   num_devices=N_CORES)

    x_d = nc.dram_tensor("x", [BS, C, HW], fp, kind="ExternalInput").ap()
    boost_d = nc.dram_tensor("boost", [C, 1], fp, kind="ExternalInput").ap()
    iota_d = nc.dram_tensor("iota", [128, NPAD], fp, kind="ExternalInput").ap()
    mask_d = nc.dram_tensor("mask", [BS, C, HW // 8], mybir.dt.uint8,
                            kind="ExternalOutput").ap()
    st_d = nc.dram_tensor("stats", [BS, 8], fp, kind="ExternalOutput").ap()

    from concourse import library_config

    es = ExitStack()
    with TileContext(nc) as tc, es:
        nc.gpsimd.load_library(library_config.attn)
        cpool = es.enter_context(tc.tile_pool(name="const", bufs=1))
        xpool = es.enter_context(tc.tile_pool(name="x", bufs=2))
        ypool = es.enter_context(tc.tile_pool(name="y", bufs=2))
        tpool = es.enter_context(tc.tile_pool(name="t", bufs=2))
        opool = es.enter_context(tc.tile_pool(name="o", bufs=2))
        zpool = es.enter_context(tc.tile_pool(name="z", bufs=2))
        spool = es.enter_context(tc.tile_pool(name="s", bufs=3))
        ppool = es.enter_context(tc.tile_pool(name="ps", bufs=1, space="PSUM"))

        boost_t = cpool.tile([128, 2], fp, tag="boost")
        nc.sync.dma_start(boost_t[:, 0:1], boost_d[0:128, :])
        nc.sync.dma_start(boost_t[:, 1:2], boost_d[128:256, :])
        iota_t = cpool.tile([128, NPAD], fp, tag="iota")
        nc.sync.dma_start(iota_t, iota_d)
        padval = cpool.tile([128, NPAD], fp, tag="padval")
        nc.vector.memset(padval, VALID_PAD)
        onesT = cpool.tile([128, 1], fp, tag="onesT")   # lhsT for col sums
        nc.vector.memset(onesT, 1.0)
        ones1 = cpool.tile([1, 128], fp, tag="ones1")   # lhsT for broadcast
        nc.vector.memset(ones1, 1.0)
        scr = cpool.tile([128, HW], fp, tag="scr")      # sign-output scratch
        negu0 = cpool.tile([128, 1], fp, tag="negu0")
        nc.vector.memset(negu0, -u0)

        for s in range(BS):
            xa = xpool.tile([128, HW], fp, tag="xa")
            xb = xpool.tile([128, HW], fp, tag="xb")
            nc.sync.dma_start(xa, x_d[s, 0:128, :])
            nc.sync.dma_start(xb, x_d[s, 128:256, :])

            ya = ypool.tile([128, HW], fp, tag="ya")
            yb = ypool.tile([128, HW], fp, tag="yb")
            nc.scalar.mul(ya, xa, boost_t[:, 0:1])
            nc.scalar.mul(yb, xb, boost_t[:, 1:2])

            # --- coarse count via sign-sum at u0 ---------------------------
            sgn = spool.tile([128, 2], fp, tag="sgn")
            nc.scalar.activation(scr, ya, Act.Sign, bias=negu0[:, 0:1],
                                 accum_out=sgn[:, 0:1])
            nc.scalar.activation(scr, yb, Act.Sign, bias=negu0[:, 0:1],
                                 accum_out=sgn[:, 1:2])
            psS = ppool.tile([1, 1], fp, tag="psS")
            nc.tensor.matmul(psS, onesT, sgn[:, 0:1], start=True, stop=False)
            nc.tensor.matmul(psS, onesT, sgn[:, 1:2], start=False, stop=True)

            # u1 = slope*S + icept ; u2 = u1 - c2   (packed [1,2])
            u12s = spool.tile([1, 2], fp, tag="u12s")
            nc.vector.tensor_scalar(u12s[0:1, 0:1], psS, slope, icept,
                                    op0=Alu.mult, op1=Alu.add)
            nc.vector.tensor_scalar(u12s[0:1, 1:2], u12s[0:1, 0:1], -c2, None,
                                    op0=Alu.add)
            psU = ppool.tile([128, 2], fp, tag="psU")
            nc.tensor.matmul(psU, ones1, u12s, start=True, stop=True)
            u12 = spool.tile([128, 2], fp, tag="u12")
            nc.vector.tensor_copy(u12, psU)

            # --- exact c1 and band count B ---------------------------------
            ta = tpool.tile([128, HW], fp, tag="ta")
            tb = tpool.tile([128, HW], fp, tag="tb")
            fa = tpool.tile([128, HW], mybir.dt.uint8, tag="fa")
            fb = tpool.tile([128, HW], mybir.dt.uint8, tag="fb")
            acc = spool.tile([128, 4], fp, tag="acc")
            nc.vector.tensor_scalar(ta, ya, u12[:, 0:1], None, op0=Alu.is_ge,
                                    op1=Alu.add, accum_out=acc[:, 0:1])
            nc.vector.tensor_scalar(tb, yb, u12[:, 0:1], None, op0=Alu.is_ge,
                                    op1=Alu.add, accum_out=acc[:, 1:2])
            nc.vector.scalar_tensor_tensor(fa, ya, u12[:, 1:2], ta,
                                           op0=Alu.is_ge, op1=Alu.subtract,
                                           accum_out=acc[:, 2:3])
            nc.vector.scalar_tensor_tensor(fb, yb, u12[:, 1:2], tb,
                                           op0=Alu.is_ge, op1=Alu.subtract,
                                           accum_out=acc[:, 3:4])
            psA = ppool.tile([1, 2], fp, tag="psA")     # [c1, B]
            nc.tensor.matmul(psA, onesT, acc[:, 0:4:2], start=True, stop=False)
            nc.tensor.matmul(psA, onesT, acc[:, 1:4:2], start=False, stop=True)

            # r = clamp(K - c1, 2, 508) ; P = 16r - B - 31 (>= 0)
            rP = spool.tile([1, 2], fp, tag="rP")
            nc.vector.tensor_scalar(rP[0:1, 0:1], psA[0:1, 0:1], -1.0, float(K),
                                    op0=Alu.mult, op1=Alu.add)
            nc.vector.tensor_scalar(rP[0:1, 0:1], rP[0:1, 0:1], 2.0, 508.0,
                                    op0=Alu.max, op1=Alu.min)
            nc.vector.scalar_tensor_tensor(rP[0:1, 1:2], rP[0:1, 0:1], 16.0,
                                           psA[0:1, 1:2],
                                           op0=Alu.mult, op1=Alu.subtract)
            nc.vector.tensor_scalar(rP[0:1, 1:2], rP[0:1, 1:2], -31.0, 0.0,
                                    op0=Alu.add, op1=Alu.max)
            psP = ppool.tile([128, 1], fp, tag="psP")
            nc.tensor.matmul(psP, ones1, rP[0:1, 1:2], start=True, stop=True)

            # --- zz: band values + P valid pads ---------------------------
            zz = zpool.tile([128, NPL], fp, tag="zz")
            nc.gpsimd.memset(zz, INVALID)
            nc.vector.copy_predicated(zz[:, 0:HW], fa, ya)
            nc.vector.copy_predicated(zz[:, HW:2 * HW], fb, yb)
            pm = spool.tile([128, NPAD], mybir.dt.uint8, tag="pm")
            nc.vector.tensor_scalar(pm, iota_t, psP, None, op0=Alu.is_lt)
            nc.vector.copy_predicated(zz[:, 2 * HW:], pm, padval)

            kout = spool.tile([1, 2], fp, tag="kout")
            nc.gpsimd.kth_largest(kout, zz, n_per_lane=NPL, k=510,
                                  quantile=1.0 - 1.0 / 16.0)

            psT = ppool.tile([128, 1], fp, tag="psT")
            nc.tensor.matmul(psT, ones1, kout[0:1, 1:2], start=True, stop=True)
            Tb = spool.tile([128, 1], fp, tag="Tb")
            nc.vector.tensor_copy(Tb, psT)

            # --- final mask, packed 8 bits/byte ---------------------------
            ma = opool.tile([128, HW], fp, tag="ma")
            mb = opool.tile([128, HW], fp, tag="mb")
            nc.vector.tensor_scalar(ma, ya, Tb, None, op0=Alu.is_ge)
            nc.vector.tensor_scalar(mb, yb, Tb, None, op0=Alu.is_ge)
            # pack: byte m = sum_i mask[8m+i] * 2^i  (little-endian bits)
            pk0 = opool.tile([128, 256], fp, tag="pk0")
            pk1 = opool.tile([128, 256], fp, tag="pk1")
            for h, m in ((0, ma), (1, mb)):
                lo = 128 * h
                cur, alt = pk0[:, lo:lo + 128], pk1[:, lo:lo + 128]
                nc.vector.tensor_scalar(cur, m[:, 0:HW:8], 1.0, None,
                                        op0=Alu.mult)
                for i in range(1, 8):
                    nc.vector.scalar_tensor_tensor(alt, m[:, i:HW:8],
                                                   float(1 << i), cur,
                                                   op0=Alu.mult, op1=Alu.add)
                    cur, alt = alt, cur
            # 7 swaps -> final accumulation lives in pk1
            u8 = opool.tile([128, 256], mybir.dt.uint8, tag="u8")
            nc.vector.tensor_copy(u8, pk1)
            nc.sync.dma_start(mask_d[s, 0:128, :], u8[:, 0:128])
            nc.sync.dma_start(mask_d[s, 128:256, :], u8[:, 128:256])

            nc.sync.dma_start(st_d[s:s + 1, 2:4], rP)        # r, P
            nc.sync.dma_start(st_d[s:s + 1, 4:6], kout)      # lerp, T

    nc.compile()
    return nc


class _Exec:
    """Cached PJRT executable for the SPMD bass program.

    Replicates concourse.bass2jax.run_bass_via_pjrt's multi-core path but
    builds the jitted shard_map once and reuses it, and assembles global
    inputs without host-side concatenation copies.
    """

    def __init__(self, nc, boost: np.ndarray):
        import jax
        import jax.numpy as jnp
        from jax.experimental.shard_map import shard_map
        from jax.sharding import Mesh, NamedSharding, PartitionSpec

        import concourse.mybir as mybir
        from concourse import bass2jax

        bass2jax.install_neuronx_cc_hook()
        self.nc = nc
        if nc.dbg_callbacks:
            raise RuntimeError("dbg_callbacks unsupported on the axon client")

        partition_name = (nc.partition_id_tensor.name
                          if nc.partition_id_tensor else None)
        in_names: list[str] = []
        out_names: list[str] = []
        out_avals = []
        zero_shapes: list[tuple[tuple, np.dtype]] = []
        for alloc in nc.m.functions[0].allocations:
            if not isinstance(alloc, mybir.MemoryLocationSet):
                continue
            name = alloc.memorylocations[0].name
            if alloc.kind == "ExternalInput":
                if name != partition_name:
                    in_names.append(name)
            elif alloc.kind == "ExternalOutput":
                shape = tuple(alloc.tensor_shape)
                dtype = mybir.dt.np(alloc.dtype)
                out_names.append(name)
                out_avals.append(jax.core.ShapedArray(shape, dtype))
                zero_shapes.append(
                    ((N_CORES * shape[0], *shape[1:]), np.dtype(dtype)))
        n_params = len(in_names)
        all_names = list(in_names) + out_names
        if partition_name is not None:
            all_names.append(partition_name)

        def _body(*args):
            operands = list(args)
            if partition_name is not None:
                operands.append(bass2jax.partition_id_tensor())
            outs = bass2jax._bass_exec_p.bind(
                *operands,
                out_avals=tuple(out_avals),
                in_names=tuple(all_names),
                out_names=tuple(out_names),
                lowering_input_output_aliases=(),
                sim_require_finite=True,
                sim_require_nnan=True,
                nc=nc,
            )
            return tuple(outs)

        devices = jax.devices()[:N_CORES]
        assert len(devices) == N_CORES
        mesh = Mesh(np.asarray(devices), ("core",))
        n_outs = len(out_names)
        donate = tuple(range(n_params, n_params + n_outs))
        self.fn = jax.jit(
            shard_map(_body, mesh=mesh,
                      in_specs=(PartitionSpec("core"),) * (n_params + n_outs),
                      out_specs=(PartitionSpec("core"),) * n_outs,
                      check_rep=False),
            donate_argnums=donate,
            keep_unused=True,
        )
        self.in_names = in_names
        self.out_names = out_names
        self.sharding = NamedSharding(mesh, PartitionSpec("core"))
        # constant inputs committed to device once; reused every call
        dbg_name = nc.dbg_addr.name if nc.dbg_addr is not None else None
        consts = {
            "boost": np.tile(boost.reshape(1, C, 1),
                             (N_CORES, 1, 1)).reshape(N_CORES * C, 1),
            "iota": np.tile(
                np.arange(128 * NPAD, dtype=np.float32).reshape(1, 128, NPAD),
                (N_CORES, 1, 1)).reshape(N_CORES * 128, NPAD),
        }
        if dbg_name is not None:
            consts[dbg_name] = np.zeros((N_CORES, 2), np.uint32)
        self.d_consts = {n: jax.device_put(v, self.sharding)
                         for n, v in consts.items()}
        # donated output buffers created on device (no host upload)
        mk = [(shp, jnp.dtype(dt)) for shp, dt in zero_shapes]
        self.mkz = jax.jit(
            lambda: tuple(jnp.zeros(shp, dt) for shp, dt in mk),
            out_shardings=(self.sharding,) * n_outs,
        )

    def run(self, x_global: np.ndarray):
        """Dispatch the NEFF; returns the raw (async) jax output arrays."""
        import jax
        # zeros first: the tiny on-device fill completes before the big x
        # upload streams, avoiding concurrent compute/transfer on the
        # remote cores (suspected trigger of rare NRT exec-unit wedges).
        zeros = self.mkz()
        d_x = jax.device_put(x_global, self.sharding)   # async upload starts
        args = [d_x if n == "x" else self.d_consts[n] for n in self.in_names]
        args += list(zeros)
        outs = self.fn(*args)
        return {n: outs[i] for i, n in enumerate(self.out_names)}


def _get_program(boost: np.ndarray):
    key = boost.tobytes()
    if key not in _CACHE:
        nc = _build(boost)
        _CACHE[key] = (nc, _Exec(nc, boost))
    return _CACHE[key]


def _boost_from_duty(dutyCycle: np.ndarray) -> np.ndarray:
    # computed with jax-on-CPU to bit-match the reference's jnp.exp
    duty = np.ascontiguousarray(np.asarray(dutyCycle, dtype=np.float32))
    key = duty.tobytes()
    if key not in _BOOST_CACHE:
        import jax
        import jax.numpy as jnp
        target_density = float(K) / float(N)
        cpu = jax.devices("cpu")[0]
        with jax.default_device(cpu):
            d = jax.device_put(duty, cpu)
            boost = jnp.exp((target_density - d) * 1.0)
        _BOOST_CACHE[key] = np.asarray(boost, dtype=np.float32).reshape(C)
    return _BOOST_CACHE[key]


def _apply_mask(x4: np.ndarray, mask_bytes: np.ndarray) -> np.ndarray:
    """out = x * unpacked(mask) — fused multithreaded pass on XLA CPU."""
    global _APPLY_JIT
    import jax
    import jax.numpy as jnp
    cpu = jax.devices("cpu")[0]
    if _APPLY_JIT is None:
        def _f(x, mb):
            bits = (mb[..., None] >> jnp.arange(8, dtype=jnp.uint8)) \
                & jnp.uint8(1)
            m = bits.reshape(B_FULL, C, 32, 32)
            return x * m.astype(jnp.float32)
        _APPLY_JIT = jax.jit(_f)
    with jax.default_device(cpu):
        out = _APPLY_JIT(jax.device_put(x4, cpu), jax.device_put(mask_bytes, cpu))
        return np.asarray(out)


def _numpy_reference(x: np.ndarray, boost: np.ndarray) -> np.ndarray:
    out = np.empty_like(x)
    xf = x.reshape(B_FULL, C, HW)
    for s in range(B_FULL):
        boosted = (xf[s] * boost[:, None]).ravel()
        thr = np.partition(boosted, N - K)[N - K]
        out[s] = (xf[s] * (boosted.reshape(C, HW) >= thr)).reshape(C, 32, 32)
    return out


def kernel(x: np.ndarray, dutyCycle: np.ndarray) -> np.ndarray:
    x = np.ascontiguousarray(np.asarray(x), dtype=np.float32)
    boost = _boost_from_duty(dutyCycle)
    try:
        nc, ex = _get_program(boost)
        res = ex.run(x.reshape(B_FULL, C, HW))       # reshape is a view
        mask_bytes = np.asarray(res["mask"]).reshape(B_FULL, C, HW // 8)
        try:  # prefetch stats d2h while the host applies the mask
            res["stats"].copy_to_host_async()
        except Exception:
            pass
        out = _apply_mask(x, mask_bytes)
        stats = np.asarray(res["stats"]).reshape(B_FULL, 8)
    except Exception as e:  # safety net: keep correctness if device path dies
        import traceback
        traceback.print_exc()
        print(f"kernel: device path failed ({e!r}); numpy fallback")
        return _numpy_reference(x, boost)
    global LAST_RESULTS
    LAST_RESULTS = type("R", (), {"exec_time_ns": None, "results": res})()

    # host-side validity guard (prob ~1e-6); numpy fallback per bad sample.
    # r,P were clamped on device; clamp-bound values mark invalid samples.
    r, P = stats[:, 2], stats[:, 3]
    B = 16.0 * r - 31.0 - P
    bad = (r <= 2) | (r >= 508) | (P <= 0) | (P > 8191) | (r > B)
    if bad.any():
        if not out.flags.writeable:
            out = out.copy()
        for s in np.nonzero(bad)[0]:
            boosted = (x[s].reshape(C, HW) * boost[:, None]).ravel()
            thr = np.partition(boosted, N - K)[N - K]
            out[s] = (x[s].reshape(C, HW)
                      * (boosted.reshape(C, HW) >= thr)).reshape(C, 32, 32)
    return out


# revision 11
# speedup vs baseline: 3.3209x; 1.1465x over previous
"""KWinners2d top-k masking kernel for Trainium2 (8 NeuronCores, batch-parallel).

Device algorithm (per sample, n = 256*32*32 = 262144, k = 26214):
  boosted y = x * boost[c];  T = k-th largest of y;  mask = (y >= T).

Exact k-th largest selection on device, per sample:
  1. y = boost_c * x          (ACT, per-partition scale, exact f32 mult)
  2. c0 ~= #{y >= u0}         (ACT Sign + accumulator; +-1 error harmless)
     u0 = build-time quantile of the boosted mixture at tail prob k/n.
  3. u1 = u0 + (c0-(k-300))/(n*pdf)  so that c(u1) ~= k-300 (sub-sample-exact
     Newton step using the true mixture density).  u2 = u1 - 700/(n*pdf).
  4. exact c1 = #{y >= u1} and band count B = #{u2 <= y < u1}  (fused DVE
     tensor_scalar / scalar_tensor_tensor passes with accumulators)
  5. zz = y where in band else -1e30, plus P = 16*(k-c1) - 31 - B pad slots
     of -1e28 (valid, below band).  GPSIMD kth_largest with quantile 15/16
     then computes k_adj = (B+P-1)//16 = r-2 and returns desc[r-1] = exact
     global k-th largest T (r = k - c1 = rank of T within the band).
  6. mask bits = (y >= T), packed 8-per-byte on device (DVE strided
     scale-adds), downloaded as uint8 [BS, C, HW/8].

The pipeline is exact: every count uses exact f32 compares, the band is
guaranteed (prob < 1e-6 otherwise, checked host-side via the stats output
with a numpy fallback per offending sample) to contain rank k with
r in [2,508] so the GPSIMD heap (cap 510) suffices.

Host side applies out = x * mask (a fused multithreaded XLA-CPU pass).
Only x (128 MB) is uploaded and the 4 MB packed mask downloaded; the
jitted PJRT executable is built once and cached, so steady-state wall
time is dominated by the unavoidable x upload over the axon tunnel.
"""

import math
from contextlib import ExitStack

import numpy as np

B_FULL = 128
N_CORES = 8
BS = B_FULL // N_CORES          # samples per core
C = 256
HW = 1024                       # 32*32
N = C * HW                      # per-sample elements
K = int(round(N * 0.1))         # 26214
NPAD = 64                       # pad columns in zz
NPL = 2 * HW + NPAD             # kth_largest n_per_lane = 2112
TARGET_GAP = 300.0              # c(u1) target = K - TARGET_GAP
BAND_RANKS = 700.0              # target band width in ranks
VALID_PAD = -1.0e28             # > -1e29  -> counted valid by kth_largest
INVALID = -1.0e30               # < -1e29  -> ignored by kth_largest

_CACHE: dict[bytes, tuple] = {}   # boost bytes -> (nc, exec handle)
_BOOST_CACHE: dict[bytes, np.ndarray] = {}
_APPLY_JIT = None
TRACE = False
LAST_RESULTS = None


def _mixture_consts(boost: np.ndarray):
    """u0 with P(|mixture| tail >= u0) = K/N, and pdf at u0, for the
    boosted mixture  y ~ (1/C) sum_c N(0, boost_c^2)."""
    b = boost.astype(np.float64)
    target = K / N

    def tail(u):  # P(Y >= u)
        return float(np.mean(0.5 * np.vectorize(math.erfc)(u / (b * math.sqrt(2.0)))))

    lo, hi = 0.0, 20.0
    for _ in range(80):
        mid = 0.5 * (lo + hi)
        if tail(mid) > target:
            lo = mid
        else:
            hi = mid
    u0 = 0.5 * (lo + hi)
    pdf = float(
        np.mean(np.exp(-0.5 * (u0 / b) ** 2) / (b * math.sqrt(2.0 * math.pi)))
    )
    return u0, pdf


def _build(boost: np.ndarray):
    import concourse.bass as bass
    import concourse.mybir as mybir
    from concourse.tile import TileContext

    fp = mybir.dt.float32
    Alu = mybir.AluOpType
    Act = mybir.ActivationFunctionType

    u0, pdf = _mixture_consts(boost)
    inv = 1.0 / (N * pdf)               # value-units per rank
    slope = inv / 2.0
    icept = u0 + (N / 2.0 - K + TARGET_GAP) * inv
    c2 = BAND_RANKS * inv               # u2 = u1 - c2

    import concourse.bacc as bacc
    nc = bacc.Bacc("TRN2", target_bir_lowering=False, debug=False,
                   num_devices=N_CORES)

    x_d = nc.dram_tensor("x", [BS, C, HW], fp, kind="ExternalInput").ap()
    boost_d = nc.dram_tensor("boost", [C, 1], fp, kind="ExternalInput").ap()
    iota_d = nc.dram_tensor("iota", [128, NPAD], fp, kind="ExternalInput").ap()
    mask_d = nc.dram_tensor("mask", [BS, C, HW // 8], mybir.dt.uint8,
                            kind="ExternalOutput").ap()
    st_d = nc.dram_tensor("stats", [BS, 8], fp, kind="ExternalOutput").ap()

    from concourse import library_config

    es = ExitStack()
    with TileContext(nc) as tc, es:
        nc.gpsimd.load_library(library_config.attn)
        cpool = es.enter_context(tc.tile_pool(name="const", bufs=1))
        xpool = es.enter_context(tc.tile_pool(name="x", bufs=2))
        ypool = es.enter_context(tc.tile_pool(name="y", bufs=2))
        tpool = es.enter_context(tc.tile_pool(name="t", bufs=2))
        opool = es.enter_context(tc.tile_pool(name="o", bufs=2))
        zpool = es.enter_context(tc.tile_pool(name="z", bufs=2))
        spool = es.enter_context(tc.tile_pool(name="s", bufs=3))
        ppool = es.enter_context(tc.tile_pool(name="ps", bufs=1, space="PSUM"))

        boost_t = cpool.tile([128, 2], fp, tag="boost")
        nc.sync.dma_start(boost_t[:, 0:1], boost_d[0:128, :])
        nc.sync.dma_start(boost_t[:, 1:2], boost_d[128:256, :])
        iota_t = cpool.tile([128, NPAD], fp, tag="iota")
        nc.sync.dma_start(iota_t, iota_d)
        padval = cpool.tile([128, NPAD], fp, tag="padval")
        nc.vector.memset(padval, VALID_PAD)
        onesT = cpool.tile([128, 1], fp, tag="onesT")   # lhsT for col sums
        nc.vector.memset(onesT, 1.0)
        ones1 = cpool.tile([1, 128], fp, tag="ones1")   # lhsT for broadcast
        nc.vector.memset(ones1, 1.0)
        scr = cpool.tile([128, HW], fp, tag="scr")      # sign-output scratch
        negu0 = cpool.tile([128, 1], fp, tag="negu0")
        nc.vector.memset(negu0, -u0)

        for s in range(BS):
            xa = xpool.tile([128, HW], fp, tag="xa")
            xb = xpool.tile([128, HW], fp, tag="xb")
            nc.sync.dma_start(xa, x_d[s, 0:128, :])
            nc.sync.dma_start(xb, x_d[s, 128:256, :])

            ya = ypool.tile([128, HW], fp, tag="ya")
            yb = ypool.tile([128, HW], fp, tag="yb")
            nc.scalar.mul(ya, xa, boost_t[:, 0:1])
            nc.scalar.mul(yb, xb, boost_t[:, 1:2])

            # --- coarse count via sign-sum at u0 ---------------------------
            sgn = spool.tile([128, 2], fp, tag="sgn")
            nc.scalar.activation(scr, ya, Act.Sign, bias=negu0[:, 0:1],
                                 accum_out=sgn[:, 0:1])
            nc.scalar.activation(scr, yb, Act.Sign, bias=negu0[:, 0:1],
                                 accum_out=sgn[:, 1:2])
            psS = ppool.tile([1, 1], fp, tag="psS")
            nc.tensor.matmul(psS, onesT, sgn[:, 0:1], start=True, stop=False)
            nc.tensor.matmul(psS, onesT, sgn[:, 1:2], start=False, stop=True)

            # u1 = slope*S + icept ; u2 = u1 - c2   (packed [1,2])
            u12s = spool.tile([1, 2], fp, tag="u12s")
            nc.vector.tensor_scalar(u12s[0:1, 0:1], psS, slope, icept,
                                    op0=Alu.mult, op1=Alu.add)
            nc.vector.tensor_scalar(u12s[0:1, 1:2], u12s[0:1, 0:1], -c2, None,
                                    op0=Alu.add)
            psU = ppool.tile([128, 2], fp, tag="psU")
            nc.tensor.matmul(psU, ones1, u12s, start=True, stop=True)
            u12 = spool.tile([128, 2], fp, tag="u12")
            nc.vector.tensor_copy(u12, psU)

            # --- exact c1 and band count B ---------------------------------
            ta = tpool.tile([128, HW], fp, tag="ta")
            tb = tpool.tile([128, HW], fp, tag="tb")
            fa = tpool.tile([128, HW], mybir.dt.uint8, tag="fa")
            fb = tpool.tile([128, HW], mybir.dt.uint8, tag="fb")
            acc = spool.tile([128, 4], fp, tag="acc")
            nc.vector.tensor_scalar(ta, ya, u12[:, 0:1], None, op0=Alu.is_ge,
                                    op1=Alu.add, accum_out=acc[:, 0:1])
            nc.vector.tensor_scalar(tb, yb, u12[:, 0:1], None, op0=Alu.is_ge,
                                    op1=Alu.add, accum_out=acc[:, 1:2])
            nc.vector.scalar_tensor_tensor(fa, ya, u12[:, 1:2], ta,
                                           op0=Alu.is_ge, op1=Alu.subtract,
                                           accum_out=acc[:, 2:3])
            nc.vector.scalar_tensor_tensor(fb, yb, u12[:, 1:2], tb,
                                           op0=Alu.is_ge, op1=Alu.subtract,
                                           accum_out=acc[:, 3:4])
            psA = ppool.tile([1, 2], fp, tag="psA")     # [c1, B]
            nc.tensor.matmul(psA, onesT, acc[:, 0:4:2], start=True, stop=False)
            nc.tensor.matmul(psA, onesT, acc[:, 1:4:2], start=False, stop=True)

            # r = clamp(K - c1, 2, 508) ; P = 16r - B - 31 (>= 0)
            rP = spool.tile([1, 2], fp, tag="rP")
            nc.vector.tensor_scalar(rP[0:1, 0:1], psA[0:1, 0:1], -1.0, float(K),
                                    op0=Alu.mult, op1=Alu.add)
            nc.vector.tensor_scalar(rP[0:1, 0:1], rP[0:1, 0:1], 2.0, 508.0,
                                    op0=Alu.max, op1=Alu.min)
            nc.vector.scalar_tensor_tensor(rP[0:1, 1:2], rP[0:1, 0:1], 16.0,
                                           psA[0:1, 1:2],
                                           op0=Alu.mult, op1=Alu.subtract)
            nc.vector.tensor_scalar(rP[0:1, 1:2], rP[0:1, 1:2], -31.0, 0.0,
                                    op0=Alu.add, op1=Alu.max)
            psP = ppool.tile([128, 1], fp, tag="psP")
            nc.tensor.matmul(psP, ones1, rP[0:1, 1:2], start=True, stop=True)

            # --- zz: band values + P valid pads ---------------------------
            zz = zpool.tile([128, NPL], fp, tag="zz")
            nc.gpsimd.memset(zz, INVALID)
            nc.vector.copy_predicated(zz[:, 0:HW], fa, ya)
            nc.vector.copy_predicated(zz[:, HW:2 * HW], fb, yb)
            pm = spool.tile([128, NPAD], mybir.dt.uint8, tag="pm")
            nc.vector.tensor_scalar(pm, iota_t, psP, None, op0=Alu.is_lt)
            nc.vector.copy_predicated(zz[:, 2 * HW:], pm, padval)

            kout = spool.tile([1, 2], fp, tag="kout")
            nc.gpsimd.kth_largest(kout, zz, n_per_lane=NPL, k=510,
                                  quantile=1.0 - 1.0 / 16.0)

            psT = ppool.tile([128, 1], fp, tag="psT")
            nc.tensor.matmul(psT, ones1, kout[0:1, 1:2], start=True, stop=True)
            Tb = spool.tile([128, 1], fp, tag="Tb")
            nc.vector.tensor_copy(Tb, psT)

            # --- final mask, packed 8 bits/byte ---------------------------
            ma = opool.tile([128, HW], fp, tag="ma")
            mb = opool.tile([128, HW], fp, tag="mb")
            nc.vector.tensor_scalar(ma, ya, Tb, None, op0=Alu.is_ge)
            nc.vector.tensor_scalar(mb, yb, Tb, None, op0=Alu.is_ge)
            # pack: byte m = sum_i mask[8m+i] * 2^i  (little-endian bits)
            pk0 = opool.tile([128, 256], fp, tag="pk0")
            pk1 = opool.tile([128, 256], fp, tag="pk1")
            for h, m in ((0, ma), (1, mb)):
                lo = 128 * h
                cur, alt = pk0[:, lo:lo + 128], pk1[:, lo:lo + 128]
                nc.vector.tensor_scalar(cur, m[:, 0:HW:8], 1.0, None,
                                        op0=Alu.mult)
                for i in range(1, 8):
                    nc.vector.scalar_tensor_tensor(alt, m[:, i:HW:8],
                                                   float(1 << i), cur,
                                                   op0=Alu.mult, op1=Alu.add)
                    cur, alt = alt, cur
            # 7 swaps -> final accumulation lives in pk1
            u8 = opool.tile([128, 256], mybir.dt.uint8, tag="u8")
            nc.vector.tensor_copy(u8, pk1)
            nc.sync.dma_start(mask_d[s, 0:128, :], u8[:, 0:128])
            nc.sync.dma_start(mask_d[s, 128:256, :], u8[:, 128:256])

            nc.sync.dma_start(st_d[s:s + 1, 2:4], rP)        # r, P
            nc.sync.dma_start(st_d[s:s + 1, 4:6], kout)      # lerp, T

    nc.compile()
    return nc


class _Exec:
    """Cached PJRT executable for the SPMD bass program.

    Replicates concourse.bass2jax.run_bass_via_pjrt's multi-core path but
    builds the jitted shard_map once and reuses it, and assembles global
    inputs without host-side concatenation copies.
    """

    def __init__(self, nc, boost: np.ndarray):
        import jax
        import jax.numpy as jnp
        from jax.experimental.shard_map import shard_map
        from jax.sharding import Mesh, NamedSharding, PartitionSpec

        import concourse.mybir as mybir
        from concourse import bass2jax

        bass2jax.install_neuronx_cc_hook()
        self.nc = nc
        if nc.dbg_callbacks:
            raise RuntimeError("dbg_callbacks unsupported on the axon client")

        partition_name = (nc.partition_id_tensor.name
                          if nc.partition_id_tensor else None)
        in_names: list[str] = []
        out_names: list[str] = []
        out_avals = []
        zero_shapes: list[tuple[tuple, np.dtype]] = []
        for alloc in nc.m.functions[0].allocations:
            if not isinstance(alloc, mybir.MemoryLocationSet):
                continue
            name = alloc.memorylocations[0].name
            if alloc.kind == "ExternalInput":
                if name != partition_name:
                    in_names.append(name)
            elif alloc.kind == "ExternalOutput":
                shape = tuple(alloc.tensor_shape)
                dtype = mybir.dt.np(alloc.dtype)
                out_names.append(name)
                out_avals.append(jax.core.ShapedArray(shape, dtype))
                zero_shapes.append(
                    ((N_CORES * shape[0], *shape[1:]), np.dtype(dtype)))
        n_params = len(in_names)
        all_names = list(in_names) + out_names
        if partition_name is not None:
            all_names.append(partition_name)

        def _body(*args):
            operands = list(args)
            if partition_name is not None:
                operands.append(bass2jax.partition_id_tensor())
            outs = bass2jax._bass_exec_p.bind(
                *operands,
                out_avals=tuple(out_avals),
                in_names=tuple(all_names),
                out_names=tuple(out_names),
                lowering_input_output_aliases=(),
                sim_require_finite=True,
                sim_require_nnan=True,
                nc=nc,
            )
            return tuple(outs)

        devices = jax.devices()[:N_CORES]
        assert len(devices) == N_CORES
        mesh = Mesh(np.asarray(devices), ("core",))
        n_outs = len(out_names)
        donate = tuple(range(n_params, n_params + n_outs))
        self.fn = jax.jit(
            shard_map(_body, mesh=mesh,
                      in_specs=(PartitionSpec("core"),) * (n_params + n_outs),
                      out_specs=(PartitionSpec("core"),) * n_outs,
                      check_rep=False),
            donate_argnums=donate,
            keep_unused=True,
        )
        self.in_names = in_names
        self.out_names = out_names
        self.sharding = NamedSharding(mesh, PartitionSpec("core"))
        # constant inputs committed to device once; reused every call
        dbg_name = nc.dbg_addr.name if nc.dbg_addr is not None else None
        consts = {
            "boost": np.tile(boost.reshape(1, C, 1),
                             (N_CORES, 1, 1)).reshape(N_CORES * C, 1),
            "iota": np.tile(
                np.arange(128 * NPAD, dtype=np.float32).reshape(1, 128, NPAD),
                (N_CORES, 1, 1)).reshape(N_CORES * 128, NPAD),
        }
        if dbg_name is not None:
            consts[dbg_name] = np.zeros((N_CORES, 2), np.uint32)
        self.d_consts = {n: jax.device_put(v, self.sharding)
                         for n, v in consts.items()}
        # donated output buffers created on device (no host upload)
        mk = [(shp, jnp.dtype(dt)) for shp, dt in zero_shapes]
        self.mkz = jax.jit(
            lambda: tuple(jnp.zeros(shp, dt) for shp, dt in mk),
            out_shardings=(self.sharding,) * n_outs,
        )

    def run(self, x_global: np.ndarray):
        """Dispatch the NEFF; returns the raw (async) jax output arrays."""
        import jax
        # zeros first: the tiny on-device fill completes before the big x
        # upload streams, avoiding concurrent compute/transfer on the
        # remote cores (suspected trigger of rare NRT exec-unit wedges).
        zeros = self.mkz()
        d_x = jax.device_put(x_global, self.sharding)   # async upload starts
        args = [d_x if n == "x" else self.d_consts[n] for n in self.in_names]
        args += list(zeros)
        outs = self.fn(*args)
        return {n: outs[i] for i, n in enumerate(self.out_names)}


def _get_program(boost: np.ndarray):
    key = boost.tobytes()
    if key not in _CACHE:
        nc = _build(boost)
        _CACHE[key] = (nc, _Exec(nc, boost))
    return _CACHE[key]


def _boost_from_duty(dutyCycle: np.ndarray) -> np.ndarray:
    # computed with jax-on-CPU to bit-match the reference's jnp.exp
    duty = np.ascontiguousarray(np.asarray(dutyCycle, dtype=np.float32))
    key = duty.tobytes()
    if key not in _BOOST_CACHE:
        import jax
        import jax.numpy as jnp
        target_density = float(K) / float(N)
        cpu = jax.devices("cpu")[0]
        with jax.default_device(cpu):
            d = jax.device_put(duty, cpu)
            boost = jnp.exp((target_density - d) * 1.0)
        _BOOST_CACHE[key] = np.asarray(boost, dtype=np.float32).reshape(C)
    return _BOOST_CACHE[key]


def _apply_mask(x4: np.ndarray, mask_bytes: np.ndarray) -> np.ndarray:
    """out = x * unpacked(mask) — single fused XLA-CPU pass."""
    global _APPLY_JIT
    import jax
    import jax.numpy as jnp
    cpu = jax.devices("cpu")[0]
    if _APPLY_JIT is None:
        def _f(x, mb):
            bits = (mb[..., None] >> jnp.arange(8, dtype=jnp.uint8)) \
                & jnp.uint8(1)
            m = bits.reshape(B_FULL, C, 32, 32)
            return x * m.astype(jnp.float32)
        _APPLY_JIT = jax.jit(_f)
    with jax.default_device(cpu):
        out = _APPLY_JIT(jax.device_put(x4, cpu), jax.device_put(mask_bytes, cpu))
        return np.asarray(out)


def _numpy_reference(x: np.ndarray, boost: np.ndarray) -> np.ndarray:
    out = np.empty_like(x)
    xf = x.reshape(B_FULL, C, HW)
    for s in range(B_FULL):
        boosted = (xf[s] * boost[:, None]).ravel()
        thr = np.partition(boosted, N - K)[N - K]
        out[s] = (xf[s] * (boosted.reshape(C, HW) >= thr)).reshape(C, 32, 32)
    return out


def kernel(x: np.ndarray, dutyCycle: np.ndarray) -> np.ndarray:
    x = np.ascontiguousarray(np.asarray(x), dtype=np.float32)
    boost = _boost_from_duty(dutyCycle)
    try:
        nc, ex = _get_program(boost)
        res = ex.run(x.reshape(B_FULL, C, HW))       # reshape is a view
        mask_bytes = np.asarray(res["mask"]).reshape(B_FULL, C, HW // 8)
        try:  # prefetch stats d2h while the host applies the mask
            res["stats"].copy_to_host_async()
        except Exception:
            pass
        out = _apply_mask(x, mask_bytes)
        stats = np.asarray(res["stats"]).reshape(B_FULL, 8)
    except Exception as e:  # safety net: keep correctness if device path dies
        import traceback
        traceback.print_exc()
        print(f"kernel: device path failed ({e!r}); numpy fallback")
        return _numpy_reference(x, boost)
    global LAST_RESULTS
    LAST_RESULTS = type("R", (), {"exec_time_ns": None, "results": res})()

    # host-side validity guard (prob ~1e-6); numpy fallback per bad sample.
    # r,P were clamped on device; clamp-bound values mark invalid samples.
    r, P = stats[:, 2], stats[:, 3]
    B = 16.0 * r - 31.0 - P
    bad = (r <= 2) | (r >= 508) | (P <= 0) | (P > 8191) | (r > B)
    if bad.any():
        if not out.flags.writeable:
            out = out.copy()
        for s in np.nonzero(bad)[0]:
            boosted = (x[s].reshape(C, HW) * boost[:, None]).ravel()
            thr = np.partition(boosted, N - K)[N - K]
            out[s] = (x[s].reshape(C, HW)
                      * (boosted.reshape(C, HW) >= thr)).reshape(C, 32, 32)
    return out
